# revision 7
# baseline (speedup 1.0000x reference)
"""Mesa-layer memory kernel for Trainium2 (8 NeuronCores, data-parallel over B).

Math: the reference's T-step Sherman-Morrison / discounted-accumulation
recurrence has a closed form,
    R_final = (I + K^T K)^{-1}            (eps term is O(1e-6) relative)
    S_final^T = K^T diag(c) V,   c_t = prod_{s>t} gamma_s
so per memory b the output is
    out_b = Q_b @ (R_b @ S_b^T).
R is computed with Newton-Schulz iterations (pure matmuls; I + K^T K is
well-conditioned here, ~6 iterations reach fp32 accuracy; we run 8).
c is computed in log space with a free-dim prefix-sum scan.

Each core owns B/8 = 8 independent memories; no cross-core communication.
"""

import numpy as np

B, T, DK, DV, NQ = 64, 2048, 128, 128, 2048
NCORES = 8
BPC = B // NCORES          # memories per core
P = 128                    # partitions
NCH = T // P               # 16 chunks of 128 timesteps
NS_ITERS = 8
GCLAMP = 1e-30             # gamma clamp before log (exact-0 gammas)


def build_nc(ns_iters=NS_ITERS):
    import concourse.mybir as mybir
    import concourse.tile as tile
    from concourse import bacc
    from concourse.masks import make_identity

    fp32 = mybir.dt.float32
    AF = mybir.ActivationFunctionType
    OP = mybir.AluOpType
    AX = mybir.AxisListType

    # Bacc (not raw Bass): its compile() pass splits multi-sem sync waits to
    # the 1-wait-per-instruction limit the TRN2 encodings require.
    nc = bacc.Bacc(trn_type="TRN2", target_bir_lowering=False, debug=False)
    keys = nc.dram_tensor("keys", [BPC, T, DK], fp32, kind="ExternalInput").ap()
    values = nc.dram_tensor("values", [BPC, T, DV], fp32, kind="ExternalInput").ap()
    gammas = nc.dram_tensor("gammas", [BPC, T], fp32, kind="ExternalInput").ap()
    queries = nc.dram_tensor("queries", [BPC, NQ, DK], fp32, kind="ExternalInput").ap()
    out = nc.dram_tensor("out", [BPC, NQ, DV], fp32, kind="ExternalOutput").ap()

    with tile.TileContext(nc) as tc:
        const = tc.alloc_tile_pool(name="const", bufs=1)
        gam = tc.alloc_tile_pool(name="gam", bufs=1)
        kvp = tc.alloc_tile_pool(name="kvp", bufs=2)
        qp = tc.alloc_tile_pool(name="qp", bufs=2)
        qtp = tc.alloc_tile_pool(name="qtp", bufs=2)
        small = tc.alloc_tile_pool(name="small", bufs=1)
        xs = tc.alloc_tile_pool(name="xs", bufs=2)
        outp = tc.alloc_tile_pool(name="outp", bufs=2)
        ps_as = tc.alloc_tile_pool(name="ps_as", bufs=2, space="PSUM")
        ps_mm = tc.alloc_tile_pool(name="ps_mm", bufs=4, space="PSUM")

        ident = const.tile([P, P], fp32)
        make_identity(nc, ident)

        # ---- phase 0: per-memory suffix cumprod of gammas (log space) ----
        g_sb = gam.tile([BPC, T], fp32)
        nc.sync.dma_start(g_sb[:], gammas[:, :])
        c8 = gam.tile([BPC, T], fp32)          # also used as the scan's zero operand
        nc.vector.memset(c8[:], 0.0)
        nc.vector.tensor_scalar_max(g_sb[:], g_sb[:], GCLAMP)
        nc.scalar.activation(g_sb[:], g_sb[:], AF.Ln)
        incl = gam.tile([BPC, T], fp32)
        # joiner: make DVE observe the ACT (Ln) dependency here, so the scan
        # below needs no sync-wait commands of its own (walrus limits them
        # for the STT instruction encoding).
        joiner = gam.tile([BPC, 1], fp32)
        nc.vector.tensor_copy(out=joiner[:], in_=g_sb[:, 0:1])
        nc.vector.tensor_tensor_scan(
            incl[:], g_sb[:], c8[:], 0.0, OP.add, OP.add
        )
        # c = exp(total - incl);  total = inclusive sum at t = T-1
        nc.scalar.activation(
            c8[:], incl[:], AF.Exp, bias=incl[:, T - 1 : T], scale=-1.0
        )
        # PE warmup op: absorb the gpsimd(identity) dependency into PE's
        # observed clock so later matmuls carry fewer sync waits (walrus
        # limits waits per matmul encoding).
        ps_warm = ps_mm.tile([BPC, BPC], fp32, tag="mm", name="ps_warm")
        nc.tensor.transpose(ps_warm[:], ident[:BPC, :BPC], ident[:BPC, :BPC])

        # transpose to [t-within-chunk (partitions), chunk, memory]
        c_t = gam.tile([P, NCH, BPC], fp32)
        for j in range(NCH):
            ps_ct = ps_mm.tile([P, BPC], fp32, tag="mm", name=f"ps_ct{j}")
            nc.tensor.transpose(ps_ct[:], c8[:, j * P : (j + 1) * P], ident[:BPC, :BPC])
            nc.vector.tensor_copy(out=c_t[:, j, :], in_=ps_ct[:])

        # ---- per-memory state tiles ----
        A_sb = [small.tile([P, P], fp32, tag=f"A{i}", name=f"A{i}") for i in range(BPC)]
        ST_sb = [small.tile([P, P], fp32, tag=f"S{i}", name=f"S{i}") for i in range(BPC)]
        Phi_sb = [small.tile([P, P], fp32, tag=f"P{i}", name=f"Phi{i}") for i in range(BPC)]
        rs_sb = [small.tile([P, 1], fp32, tag=f"r{i}", name=f"rs{i}") for i in range(BPC)]
        X_cur = [None] * BPC

        # ---- phase 1: load K/V, form A = K^T K (+I), S^T = K^T diag(c) V ----
        for i in range(BPC):
            kv = kvp.tile([P, NCH, 2 * P], fp32, tag="kv", name=f"kv{i}")
            nc.sync.dma_start(
                kv[:, :, 0:DK], keys[i].rearrange("(j p) k -> p j k", p=P)
            )
            nc.sync.dma_start(
                kv[:, :, DK : 2 * DK], values[i].rearrange("(j p) k -> p j k", p=P)
            )

            # scale V rows by suffix-cumprod weights (ScalarE, per-partition scale)
            for j in range(NCH):
                nc.scalar.activation(
                    kv[:, j, DK : 2 * DK],
                    kv[:, j, DK : 2 * DK],
                    AF.Copy,
                    scale=c_t[:, j, i : i + 1],
                )

            ps = ps_as.tile([P, 2 * P], fp32, tag="as", name=f"ps_as{i}")
            for j in range(NCH):
                nc.tensor.matmul(
                    ps[:],
                    kv[:, j, 0:DK],
                    kv[:, j, :],
                    start=(j == 0),
                    stop=(j == NCH - 1),
                )

            # A = K^T K + I ; S^T = K^T diag(c) V ; Jacobi-style NS init
            nc.vector.tensor_tensor(A_sb[i][:], ps[:, 0:P], ident[:], OP.add)
            nc.vector.tensor_copy(out=ST_sb[i][:], in_=ps[:, P : 2 * P])
            nc.vector.tensor_reduce(
                rs_sb[i][:], A_sb[i][:], AX.X, OP.add, apply_absolute_value=True
            )
            nc.vector.reciprocal(rs_sb[i][:], rs_sb[i][:])
            x0 = xs.tile([P, P], fp32, tag=f"X{i}", name=f"X{i}_0")
            nc.scalar.activation(x0[:], ident[:], AF.Copy, scale=rs_sb[i][:])
            X_cur[i] = x0

        # ---- phase 2: Newton-Schulz iterations, batched across memories ----
        for it in range(ns_iters):
            ps_ax = []
            for i in range(BPC):
                p1 = ps_mm.tile([P, P], fp32, tag="mm", name=f"ps_ax{it}_{i}")
                nc.tensor.matmul(p1[:], A_sb[i][:], X_cur[i][:])  # A X (A sym)
                ps_ax.append(p1)
            ax_sb = []
            for i in range(BPC):
                t1 = xs.tile([P, P], fp32, tag=f"AX{i}", name=f"ax{it}_{i}")
                nc.vector.tensor_copy(out=t1[:], in_=ps_ax[i][:])
                ax_sb.append(t1)
            ps_x2 = []
            for i in range(BPC):
                p2 = ps_mm.tile([P, P], fp32, tag="mm", name=f"ps_xax{it}_{i}")
                nc.tensor.matmul(p2[:], X_cur[i][:], ax_sb[i][:])  # X (A X)  (X sym)
                ps_x2.append(p2)
            for i in range(BPC):
                xn = xs.tile([P, P], fp32, tag=f"X{i}", name=f"X{i}_{it + 1}")
                nc.vector.scalar_tensor_tensor(
                    xn[:], X_cur[i][:], 2.0, ps_x2[i][:], OP.mult, OP.subtract
                )
                X_cur[i] = xn

        # ---- phase 3: Phi = R @ S^T ----
        for i in range(BPC):
            ps_phi = ps_mm.tile([P, P], fp32, tag="mm", name=f"ps_phi{i}")
            nc.tensor.matmul(ps_phi[:], X_cur[i][:], ST_sb[i][:])
            nc.vector.tensor_copy(out=Phi_sb[i][:], in_=ps_phi[:])

        # ---- phase 4: out = Q @ Phi; load Q, PE-transpose chunks, matmul ----
        for i in range(BPC):
            q_sb = qp.tile([P, NCH, DK], fp32, tag="q", name=f"q{i}")
            nc.sync.dma_start(q_sb[:], queries[i].rearrange("(j p) k -> p j k", p=P))
            qt = qtp.tile([P, NCH, P], fp32, tag="qt", name=f"qt{i}")
            for j in range(NCH):
                ps_qt = ps_mm.tile([P, P], fp32, tag="mm", name=f"ps_qt{i}_{j}")
                nc.tensor.transpose(ps_qt[:], q_sb[:, j, :], ident[:])
                nc.vector.tensor_copy(out=qt[:, j, :], in_=ps_qt[:])
            o_sb = outp.tile([P, NCH, DV], fp32, tag="o", name=f"o{i}")
            for j in range(NCH):
                ps_o = ps_mm.tile([P, P], fp32, tag="mm", name=f"ps_o{i}_{j}")
                nc.tensor.matmul(ps_o[:], qt[:, j, :], Phi_sb[i][:])
                nc.vector.tensor_copy(out=o_sb[:, j, :], in_=ps_o[:])
            nc.sync.dma_start(out[i].rearrange("(j p) v -> p j v", p=P), o_sb[:])

        for pool in (ps_mm, ps_as, outp, xs, small, qtp, qp, kvp, gam, const):
            pool.release()

    if not nc.is_finalized():
        nc.finalize()
    return nc


def kernel(**inputs) -> np.ndarray:
    keys = np.ascontiguousarray(inputs["keys"], dtype=np.float32)
    values = np.ascontiguousarray(inputs["values"], dtype=np.float32)
    gammas = np.ascontiguousarray(inputs["gammas"], dtype=np.float32)
    queries = np.ascontiguousarray(inputs["queries"], dtype=np.float32)

    from concourse.bass_utils import run_bass_kernel_spmd

    nc = build_nc()
    in_maps = []
    for m in range(NCORES):
        s = slice(m * BPC, (m + 1) * BPC)
        in_maps.append(
            {
                "keys": keys[s],
                "values": values[s],
                "gammas": gammas[s],
                "queries": queries[s],
            }
        )
    res = run_bass_kernel_spmd(nc, in_maps, core_ids=list(range(NCORES)))
    return np.concatenate([res.results[m]["out"] for m in range(NCORES)], axis=0)
